# revision 6
# baseline (speedup 1.0000x reference)
"""Trainium2 Bass kernel v2 for the 7-layer binarized CNN (nn_MCNET).

Data parallel over 8 cores (8 images each). Per core:
- L0 (3->4, fp32 input): 32 row-bands x 8 rows stacked on 128 partitions
  (block-diagonal weights), 9 accumulating fp32 matmuls; ACT Sign -> bf16,
  DVE 2x2 maxpool writes fp8 directly into A1's banded layout.
- L1..L6: fp8e4 DoubleRow matmuls. Activations live in per-layer banded
  buffers A_l: G bands x cin channels on 128 partitions, rows contiguous at
  stride 127 (127 % 16 == 15, so tap pairs (ki,kj)->(ki+1,kj+1) have ktile
  stride 128, a legal DoubleRow step). Tap-paired layers run 6 DoubleRow
  passes instead of 9; L5 (cin=64) k-splits channels across two 4320-byte
  slabs instead. Band nesting is chosen so every PSUM evacuation is an
  identity-partition ACT/DVE op (sign == clip for the even-integer sums),
  and inter-band halos are ONE contiguous partition-shifted SBUF DMA per
  layer. All DMAs issue from the SP (sync) engine -> HWDGE.
"""
import sys, os, dataclasses
sys.path.insert(0, '/opt/trn_rl_repo')
import numpy as np

CH = [(3, 4), (4, 8), (8, 16), (16, 32), (32, 64), (64, 32), (32, 2)]
HIN = [256, 127, 125, 123, 121, 119, 117]
HOUT = [h - 2 for h in HIN]
G = [32, 32, 16, 8, 4, 4, 4]         # in-bands per layer
BB = [8, 4, 8, 16, 32, 32, 32]       # nominal in-band rows
WP = 127                              # fp8 row stride (127 % 16 == 15)
SL5 = 4320                            # A5 slab stride (34*127=4318 -> pad to %16)
NIMG = 8
# tap pairs for t2 (tap-paired DoubleRow): ktile delta = 127*dki + dkj = 128
PAIRS = [((0, 0), (1, 1)), ((0, 1), (1, 2)), ((1, 0), (2, 1)),
         ((0, 2), None), ((2, 0), None), ((2, 2), None)]
# weight block column offsets in WF8
NB = [0, 12, 12, 12, 12, 9, 6]        # lhsT blocks per layer (l1..l6 used)
OFF = {}
_c = 0
for _l in range(1, 7):
    OFF[_l] = _c
    _c += NB[_l] * (256 if _l < 6 else 32)
WF8_COLS = _c
A_ROWS = [0, 6, 10, 18, 34, 0, 34]    # stored rows per band (A5 special)
A_COLS = [0] + [A_ROWS[l] * WP + 384 for l in range(1, 7)]
A_COLS[5] = 2 * SL5 + 384


def build_program():
    import concourse.bass as bass
    import concourse.mybir as mybir
    dt = mybir.dt
    AF = mybir.ActivationFunctionType
    PM = mybir.MatmulPerfMode
    ALU = mybir.AluOpType

    nc = bass.Bass("TRN2", target_bir_lowering=False)
    x = nc.dram_tensor("x", (NIMG, 3, 256, 256), dt.float32, kind="ExternalInput")
    w0f = nc.dram_tensor("w0f", (96, 9 * 128), dt.float32, kind="ExternalInput")
    wf8 = nc.dram_tensor("wf8", (128, WF8_COLS), dt.float8e4, kind="ExternalInput")
    y = nc.dram_tensor("y", (NIMG, 2 * 115 * 115), dt.float32, kind="ExternalOutput")

    ctxs = []
    def alloc(cm):
        ctxs.append(cm)
        return cm.__enter__()

    WT0 = alloc(nc.sbuf_tensor("WT0", [128, 9 * 128], dt.float32))
    WF8 = alloc(nc.sbuf_tensor("WF8", [128, WF8_COLS], dt.float8e4))
    A0 = alloc(nc.sbuf_tensor("A0", [128, 2 * 10 * 256], dt.float32))
    A = [None] * 7
    for l in range(1, 7):
        A[l] = alloc(nc.sbuf_tensor(f"A{l}", [128, A_COLS[l]], dt.float8e4))
    T0B = alloc(nc.sbuf_tensor("T0B", [128, 2032], dt.bfloat16))
    T1B = alloc(nc.sbuf_tensor("T1B", [128, 4 * 254], dt.bfloat16))
    OUTB = alloc(nc.sbuf_tensor("OUTB", [128, 32 * 115], dt.float32))
    P = [alloc(nc.psum_tensor(f"P{i}", [128, 2048], dt.float32)) for i in range(2)]
    sem = {n: alloc(nc.semaphore(name=n)) for n in
           ['sdma', 'spe', 'sact', 'sdve', 'sgp', 'sin0', 'sin1', 'swf',
            'sh1', 'sh2', 'sh3', 'sh4', 'sh5', 'sh6', 'sout']}

    def walk(E, me):
        cnt = {'dma': 0, 'pe': 0, 'act': 0, 'dve': 0, 'gp': 0}
        last_wait = {}

        def wait(eng, semn, val):
            if val is None or val <= 0:
                return
            k = (eng, semn)
            if last_wait.get(k, -1) >= val:
                return
            last_wait[k] = val
            if eng == me:
                E.wait_ge(sem[semn], val)

        def emit(eng, fn):
            if eng == me:
                return fn()
            return None

        def inc(inst, semn, v):
            if inst is not None:
                inst.then_inc(sem[semn], v)

        # ---- init: memsets ----
        # A0: only band 31's rows 8,9 are never DMA-written (both slots).
        i = emit('gp', lambda: nc.gpsimd.memset(A0[64:96, 2048:2560], 0.0))
        cnt['gp'] += 1
        inc(i, 'sgp', 1)
        i = emit('gp', lambda: nc.gpsimd.memset(A0[64:96, 4608:5120], 0.0))
        cnt['gp'] += 1
        inc(i, 'sgp', 1)
        for l in range(1, 7):
            i = emit('gp', lambda l=l:
                     nc.gpsimd.memset(A[l][0:128, 0:A_COLS[l]], 0.0))
            cnt['gp'] += 1
            inc(i, 'sgp', 1)
        NMEMSET = cnt['gp']
        MS_A = {0: 2, 1: 3, 2: 4, 3: 5, 4: 6, 5: 7, 6: 8}  # sgp count when A_l ready
        # ---- weight DMAs (separate sems so partial waits are race-free) ----
        i = emit('sp', lambda: nc.sync.dma_start(WT0[0:96, :], w0f[:]))
        inc(i, 'sdma', 16)
        i = emit('sp', lambda: nc.sync.dma_start(WF8[0:128, :], wf8[:]))
        inc(i, 'swf', 16)

        slot_free = [None, None]
        a0_free = [None, None]
        vmax_prev = None            # sdve count of prev img vmax (T0B free)
        hcnt = {l: 0 for l in range(1, 7)}   # per-halo-sem cumulative counts
        out_cnt = 0
        tile_g = 0

        def pe_tile_begin(slot, waits_other=(), layer=1):
            # common PE-tile prologue: memsets + weights + input-ready + slot
            if layer == 0:
                wait('pe', 'sgp', 2)        # A0 tail memsets
                wait('pe', 'sdma', 16)      # WT0
            else:
                wait('pe', 'sgp', MS_A[layer])   # A_layer memset done
                wait('pe', 'swf', 16)       # WF8
            for sname, v in waits_other:
                wait('pe', sname, v)
            if slot_free[slot] is not None:
                wait('pe', slot_free[slot][0], slot_free[slot][1])

        def dr_matmul(PS, psoff, lhs_col, lhs_m, rhs_buf, rhs_off, rhs_delta,
                      n, start, stop, final):
            # one DoubleRow matmul; final -> inc spe
            lstep = max(16, lhs_m)
            def mk():
                lv = WF8[0:128, lhs_col: lhs_col + lstep + lhs_m]
                lv = dataclasses.replace(lv, ap=[lv.ap[0], [lstep, 2], [1, lhs_m]])
                rv = rhs_buf[0:128, rhs_off: rhs_off + rhs_delta + n]
                rv = dataclasses.replace(rv, ap=[rv.ap[0], [rhs_delta, 2], [1, n]])
                ov = PS[0:lhs_m, psoff: psoff + n]
                return nc.tensor.matmul(ov, lv, rv, start=start, stop=stop,
                                        perf_mode=PM.DoubleRow)
            i = emit('pe', mk)
            if final:
                cnt['pe'] += 1
                inc(i, 'spe', 1)
            return i

        def evac(eng, PS, nchunks, nlast, dstbuf, dstoff, dst_ms, mpart,
                 sdma_guard):
            # evacuate psum chunks [512-strided, 508(or nlast) wide] -> dst
            # contiguous; sign/clip. eng in ('act','dve').
            wait(eng, 'spe', cnt['pe'])
            if dst_ms is not None:
                wait(eng, 'sgp', dst_ms)
            if sdma_guard is not None:
                wait(eng, sdma_guard[0], sdma_guard[1])
            total = 508 * (nchunks - 1) + nlast
            def mk():
                sv = PS[0:mpart, 0:(nchunks - 1) * 512 + nlast]
                sv = dataclasses.replace(sv, ap=[sv.ap[0], [512, nchunks], [1, 508]]) \
                    if nchunks > 1 else dataclasses.replace(sv, ap=[sv.ap[0], [1, nlast]])
                dv = dstbuf[0:mpart, dstoff: dstoff + total]
                if eng == 'act':
                    return nc.scalar.activation(dv, sv, AF.Sign)
                return nc.vector.tensor_scalar(dv, sv, 1.0, -1.0, ALU.min, ALU.max)
            i = emit(eng, mk)
            key = 'sact' if eng == 'act' else 'sdve'
            cnt[eng] += 1
            inc(i, key, 1)
            return (key, cnt[eng])

        dma_in_done = [None] * NIMG
        in_cnt = [0, 0]

        def emit_in_dma(j):
            # input DMA for image j (slot j%2); issued one image ahead.
            # Dedicated per-slot semaphore so the wait value is race-free
            # (only this slot's DMAs are ever outstanding on it).
            aslot = j % 2
            sname = f'sin{aslot}'
            off = aslot * 2560
            wait('sp', 'sgp', 2)      # A0 tail memsets done
            if a0_free[aslot] is not None:
                wait('sp', 'spe', a0_free[aslot])
            src_main = dataclasses.replace(
                x[j], ap=[[2048, 31], [65536, 3], [256, 10], [1, 256]])
            i = emit('sp', lambda src_main=src_main, off=off:
                     nc.sync.dma_start(A0[0:93, off:off + 2560], src_main))
            in_cnt[aslot] += 1
            inc(i, sname, 16)
            i = emit('sp', lambda j=j, off=off:
                     nc.sync.dma_start(A0[93:96, off:off + 2048], x[j, :, 248:256, :]))
            in_cnt[aslot] += 1
            inc(i, sname, 16)
            dma_in_done[j] = (sname, in_cnt[aslot])

        for img in range(NIMG):
            # ======== input DMA prefetch (this img on img0, next img after) ====
            if img == 0:
                emit_in_dma(0)
            if img + 1 < NIMG:
                emit_in_dma(img + 1)
            dma_in = dma_in_done[img]
            aslot = img % 2
            off = aslot * 2560

            # ======== L0: fp32 conv, 32 bands x 8 rows ========
            slot = tile_g % 2; tile_g += 1
            PS = P[slot]
            pe_tile_begin(slot, waits_other=[(dma_in[0], 16 * dma_in[1])], layer=0)
            l0_chunk_pe = []
            for c in range(4):
                for tap in range(9):
                    ki, kj = tap // 3, tap % 3
                    rbase = off + (2 * c + ki) * 256 + kj
                    def mk(c=c, rbase=rbase, tap=tap, PS=PS):
                        lhsT = WT0[0:96, tap * 128: tap * 128 + 128]
                        rv = A0[0:96, rbase: rbase + 256 + 254]
                        rv = dataclasses.replace(rv, ap=[rv.ap[0], [256, 2], [1, 254]])
                        ov = PS[0:128, c * 512: c * 512 + 508]
                        return nc.tensor.matmul(ov, lhsT, rv,
                                                start=(tap == 0), stop=(tap == 8))
                    i = emit('pe', mk)
                    if tap == 8:
                        cnt['pe'] += 1
                        inc(i, 'spe', 1)
                l0_chunk_pe.append(cnt['pe'])
            a0_free[aslot] = cnt['pe']

            # per-chunk pool pipeline: sign(c) -> vmax(c) -> hmax(c)
            vmax_cur = []
            for c in range(4):
                wait('act', 'spe', l0_chunk_pe[c])
                if vmax_prev is not None:
                    wait('act', 'sdve', vmax_prev[c])
                def mksgn(PS=PS, c=c):
                    return nc.scalar.activation(
                        T0B[0:128, c * 508: c * 508 + 508],
                        PS[0:128, c * 512: c * 512 + 508], AF.Sign)
                i = emit('act', mksgn)
                cnt['act'] += 1
                inc(i, 'sact', 1)
                if c == 3:
                    slot_free[slot] = ('sact', cnt['act'])
                # DVE vmax(c): rows 2c,2c+1 -> T1B row c
                wait('dve', 'sact', cnt['act'])
                def mkv(c=c):
                    a = T0B[0:128, c * 508: c * 508 + 254]
                    b = T0B[0:128, c * 508 + 254: c * 508 + 508]
                    return nc.vector.tensor_max(
                        T1B[0:128, c * 254: c * 254 + 254], a, b)
                i = emit('dve', mkv)
                cnt['dve'] += 1
                inc(i, 'sdve', 1)
                vmax_cur.append(cnt['dve'])
                # DVE hmax(c) -> A1 row c
                wait('dve', 'sdve', cnt['dve'])   # RAW on T1B
                if c == 0:
                    wait('dve', 'sgp', MS_A[1])
                    wait('dve', 'sh1', hcnt[1])
                def mkh(c=c):
                    sv = T1B[0:128, c * 254: c * 254 + 254]
                    a = dataclasses.replace(sv, ap=[sv.ap[0], [2, 127]])
                    b = dataclasses.replace(sv, offset=sv.offset + 1,
                                            ap=[sv.ap[0], [2, 127]])
                    return nc.vector.tensor_max(
                        A[1][0:128, c * WP: c * WP + 127], a, b)
                i = emit('dve', mkh)
                cnt['dve'] += 1
                inc(i, 'sdve', 1)
                if c == 1:
                    hmax01 = cnt['dve']
            hmax_all = cnt['dve']
            vmax_prev = vmax_cur

            # halo A1 (SP): bands g+1 rows 0:2 -> band g rows 4:6
            wait('sp', 'sdve', hmax01)
            i = emit('sp', lambda: nc.sync.dma_start(
                A[1][0:124, 4 * WP: 6 * WP], A[1][4:128, 0:2 * WP]))
            hcnt[1] += 16
            inc(i, 'sh1', 16)
            halo1 = ('sh1', hcnt[1])

            # ======== L1: G32 -> 16 out-bands, 2 phase-chunks ========
            slot = tile_g % 2; tile_g += 1
            PS = P[slot]
            pe_tile_begin(slot, waits_other=[halo1, ('sdve', hmax_all)])
            for p, (tA, tB) in enumerate(PAIRS):
                for ph in range(2):
                    kiA, kjA = tA
                    delta = 128 if tB else 16
                    dr_matmul(PS, ph * 512, OFF[1] + (p * 2 + ph) * 256, 128,
                              A[1], kiA * WP + kjA, delta, 508,
                              start=(p == 0), stop=(p == 5),
                              final=(p == 5 and ph == 1))
            # evac (DVE) -> A2 rows 0..8
            efree = evac('dve', PS, 2, 508, A[2], 0, MS_A[2], 128,
                         ('sh2', hcnt[2]) if hcnt[2] else None)
            l1_evac = efree
            slot_free[slot] = efree
            # halo A2
            wait('sp', efree[0], efree[1])
            i = emit('sp', lambda: nc.sync.dma_start(
                A[2][0:120, 8 * WP: 10 * WP], A[2][8:128, 0:2 * WP]))
            hcnt[2] += 16
            inc(i, 'sh2', 16)
            halo2 = ('sh2', hcnt[2])

            # ======== L2: G16 -> 8, chunks (ph, j) ========
            slot = tile_g % 2; tile_g += 1
            PS = P[slot]
            pe_tile_begin(slot, waits_other=[l1_evac])
            for j in range(2):
                if j == 1:
                    wait('pe', halo2[0], halo2[1])
                for ph in range(2):
                    for p, (tA, tB) in enumerate(PAIRS):
                        kiA, kjA = tA
                        delta = 128 if tB else 16
                        dr_matmul(PS, (2 * ph + j) * 512,
                                  OFF[2] + (p * 2 + ph) * 256, 128,
                                  A[2], (4 * j + kiA) * WP + kjA, delta, 508,
                                  start=(p == 0), stop=(p == 5),
                                  final=(p == 5 and ph == 1 and j == 1))
            efree = evac('act', PS, 4, 508, A[3], 0, MS_A[3], 128,
                         ('sh3', hcnt[3]) if hcnt[3] else None)
            l2_evac = efree
            slot_free[slot] = efree
            wait('sp', efree[0], efree[1])
            i = emit('sp', lambda: nc.sync.dma_start(
                A[3][0:112, 16 * WP: 18 * WP], A[3][16:128, 0:2 * WP]))
            hcnt[3] += 16
            inc(i, 'sh3', 16)
            halo3 = ('sh3', hcnt[3])

            # ======== L3: G8 -> 4, two row-tiles (tau = phase) ========
            l3_evacs = []
            for tau in range(2):
                slot = tile_g % 2; tile_g += 1
                PS = P[slot]
                pe_tile_begin(slot, waits_other=[l2_evac])
                for j in range(4):
                    if j == 3:
                        wait('pe', halo3[0], halo3[1])
                    for p, (tA, tB) in enumerate(PAIRS):
                        kiA, kjA = tA
                        delta = 128 if tB else 16
                        dr_matmul(PS, j * 512, OFF[3] + (p * 2 + tau) * 256, 128,
                                  A[3], (4 * j + kiA) * WP + kjA, delta, 508,
                                  start=(p == 0), stop=(p == 5),
                                  final=(p == 5 and j == 3))
                efree = evac('dve', PS, 4, 508, A[4], tau * 16 * WP, MS_A[4], 128,
                             ('sh4', hcnt[4]) if (tau == 0 and hcnt[4]) else None)
                slot_free[slot] = efree
                l3_evacs.append(efree)
                if tau == 0:
                    wait('sp', efree[0], efree[1])
                    i = emit('sp', lambda: nc.sync.dma_start(
                        A[4][0:96, 32 * WP: 34 * WP], A[4][32:128, 0:2 * WP]))
                    hcnt[4] += 16
                    inc(i, 'sh4', 16)
                    halo4 = ('sh4', hcnt[4])

            # ======== L4: G4, tiles (h, tau) ========
            l4_t0_evacs = []
            l4_evac_last = None
            for h in range(2):
                for tau in range(2):
                    slot = tile_g % 2; tile_g += 1
                    PS = P[slot]
                    pe_tile_begin(slot, waits_other=[l3_evacs[tau]])
                    for j in range(4):
                        if j == 3:
                            if tau == 0:
                                wait('pe', l3_evacs[1][0], l3_evacs[1][1])
                            else:
                                wait('pe', halo4[0], halo4[1])
                        for p, (tA, tB) in enumerate(PAIRS):
                            kiA, kjA = tA
                            delta = 128 if tB else 16
                            dr_matmul(PS, j * 512, OFF[4] + (p * 2 + h) * 256, 128,
                                      A[4], (16 * tau + 4 * j + kiA) * WP + kjA,
                                      delta, 508,
                                      start=(p == 0), stop=(p == 5),
                                      final=(p == 5 and j == 3))
                    efree = evac('act', PS, 4, 508, A[5],
                                 h * SL5 + tau * 16 * WP, MS_A[5], 128,
                                 ('sh5', hcnt[5]) if (h == 0 and tau == 0 and hcnt[5]) else None)
                    slot_free[slot] = efree
                    l4_evac_last = efree
                    if tau == 0:
                        l4_t0_evacs.append(efree)
            # halo A5 (both slabs, 1 DMA) after (h0,t0) and (h1,t0) evacs
            wait('sp', l4_t0_evacs[1][0], l4_t0_evacs[1][1])
            def mkh5():
                sv = A[5][32:128, 0:SL5 + 2 * WP]
                sv = dataclasses.replace(sv, ap=[sv.ap[0], [SL5, 2], [1, 2 * WP]])
                dv = A[5][0:96, 32 * WP: SL5 + 34 * WP]
                dv = dataclasses.replace(dv, ap=[dv.ap[0], [SL5, 2], [1, 2 * WP]])
                return nc.sync.dma_start(dv, sv)
            i = emit('sp', mkh5)
            hcnt[5] += 16
            inc(i, 'sh5', 16)
            halo5 = ('sh5', hcnt[5])

            # ======== L5: G4, t1 (k-split slabs), 9 taps, two row-tiles ========
            l5_evacs = []
            for tau in range(2):
                slot = tile_g % 2; tile_g += 1
                PS = P[slot]
                if tau == 0:
                    pe_tile_begin(slot, waits_other=[l4_t0_evacs[1]])
                else:
                    pe_tile_begin(slot, waits_other=[l4_evac_last])
                for j in range(4):
                    if j == 3:
                        if tau == 0:
                            wait('pe', l4_evac_last[0], l4_evac_last[1])
                        else:
                            wait('pe', halo5[0], halo5[1])
                    for tap in range(9):
                        ki, kj = tap // 3, tap % 3
                        dr_matmul(PS, j * 512, OFF[5] + tap * 256, 128,
                                  A[5], (16 * tau + 4 * j + ki) * WP + kj,
                                  SL5, 508,
                                  start=(tap == 0), stop=(tap == 8),
                                  final=(tap == 8 and j == 3))
                efree = evac('dve', PS, 4, 508, A[6], tau * 16 * WP, MS_A[6], 128,
                             ('sh6', hcnt[6]) if (tau == 0 and hcnt[6]) else None)
                slot_free[slot] = efree
                l5_evacs.append(efree)
                if tau == 0:
                    wait('sp', efree[0], efree[1])
                    i = emit('sp', lambda: nc.sync.dma_start(
                        A[6][0:96, 32 * WP: 34 * WP], A[6][32:128, 0:2 * WP]))
                    hcnt[6] += 16
                    inc(i, 'sh6', 16)
                    halo6 = ('sh6', hcnt[6])

            # ======== L6: G4, M=8, two row-tiles ========
            l6_evacs = []
            for tau in range(2):
                slot = tile_g % 2; tile_g += 1
                PS = P[slot]
                pe_tile_begin(slot, waits_other=[l5_evacs[tau]])
                for j in range(4):
                    if j == 3:
                        if tau == 0:
                            wait('pe', l5_evacs[1][0], l5_evacs[1][1])
                        else:
                            wait('pe', halo6[0], halo6[1])
                    for p, (tA, tB) in enumerate(PAIRS):
                        kiA, kjA = tA
                        delta = 128 if tB else 16
                        dr_matmul(PS, j * 512, OFF[6] + p * 32, 8,
                                  A[6], (16 * tau + 4 * j + kiA) * WP + kjA,
                                  delta, 508,
                                  start=(p == 0), stop=(p == 5),
                                  final=(p == 5 and j == 3))
                wait('act', 'spe', cnt['pe'])
                if tau == 0 and out_cnt:
                    wait('act', 'sout', out_cnt)
                def mk6(PS=PS, tau=tau):
                    sv = PS[0:8, 0:3 * 512 + 508]
                    sv = dataclasses.replace(
                        sv, ap=[sv.ap[0], [512, 4], [127, 4], [1, 115]])
                    dv = OUTB[0:8, tau * 16 * 115: tau * 16 * 115 + 16 * 115]
                    return nc.scalar.activation(dv, sv, AF.Sign)
                i = emit('act', mk6)
                cnt['act'] += 1
                inc(i, 'sact', 1)
                efree = ('sact', cnt['act'])
                slot_free[slot] = efree
                l6_evacs.append(efree)

            # ======== output DMA: 8 flat per-(band,ch) planes ========
            wait('sp', l6_evacs[1][0], l6_evacs[1][1])
            for g in range(4):
                nrows = 32 if g < 3 else 19
                for c in range(2):
                    def mko(img=img, g=g, c=c, nrows=nrows):
                        sv = OUTB[2 * g + c: 2 * g + c + 1, 0: nrows * 115]
                        dv = y[img, 0:1]
                        dv = dataclasses.replace(
                            dv, offset=dv.offset + c * 13225 + 32 * g * 115,
                            ap=[[1, nrows * 115]])
                        return nc.sync.dma_start(dv, sv)
                    i = emit('sp', mko)
                    out_cnt += 16
                    inc(i, 'sout', 16)
        return cnt

    with nc.Block() as block:
        @block.tensor
        def _(E):
            walk(E, 'pe')

        @block.scalar
        def _(E):
            walk(E, 'act')

        @block.vector
        def _(E):
            walk(E, 'dve')

        @block.gpsimd
        def _(E):
            walk(E, 'gp')

        @block.sync
        def _(E):
            walk(E, 'sp')

    for cm in reversed(ctxs):
        cm.__exit__(None, None, None)
    return nc


def pack_weights(ws):
    """ws: 7 raw arrays (cout, cin, 3, 3) -> (w0f fp32, wf8 fp8)."""
    import ml_dtypes
    sws = [np.sign(w).astype(np.float32) for w in ws]
    # L0: 32 bands x 3cin on 96 partitions -> 128 out (32 bands x 4)
    w0f = np.zeros((96, 9 * 128), np.float32)
    for tap in range(9):
        ki, kj = tap // 3, tap % 3
        blk = sws[0][:, :, ki, kj].T  # (cin, cout)
        for s in range(32):
            w0f[s * 3:s * 3 + 3, tap * 128 + s * 4: tap * 128 + s * 4 + 4] = blk
    wf8 = np.zeros((128, WF8_COLS), np.float32)
    # t2 layers: 1,2,3 (phases), 4 (cout halves), 6 (plain)
    for l, nph in ((1, 2), (2, 2), (3, 2)):
        cin, cout = CH[l]
        gin = G[l]
        M = 128
        for p, (tA, tB) in enumerate(PAIRS):
            for ph in range(nph):
                col = OFF[l] + (p * 2 + ph) * 256
                for i, tap in enumerate((tA, tB)):
                    if tap is None:
                        continue
                    ki, kj = tap
                    blk = sws[l][:, :, ki, kj].T  # (cin, cout)
                    for gp_ in range(gin // 2):
                        g = 2 * gp_ + ph
                        wf8[g * cin:(g + 1) * cin,
                            col + i * M + gp_ * cout: col + i * M + (gp_ + 1) * cout] = blk
    # L4: cout halves
    cin, cout = CH[4]
    for p, (tA, tB) in enumerate(PAIRS):
        for h in range(2):
            col = OFF[4] + (p * 2 + h) * 256
            for i, tap in enumerate((tA, tB)):
                if tap is None:
                    continue
                ki, kj = tap
                blk = sws[4][32 * h:32 * h + 32, :, ki, kj].T  # (32cin, 32cout)
                for g in range(4):
                    wf8[g * 32:(g + 1) * 32,
                        col + i * 128 + g * 32: col + i * 128 + (g + 1) * 32] = blk
    # L5: t1 k-split (slab i = channels 32i..32i+32)
    for tap in range(9):
        ki, kj = tap // 3, tap % 3
        col = OFF[5] + tap * 256
        for i in range(2):
            blk = sws[5][:, 32 * i:32 * i + 32, ki, kj].T  # (32cin-half, 32cout)
            for g in range(4):
                wf8[g * 32:(g + 1) * 32,
                    col + i * 128 + g * 32: col + i * 128 + (g + 1) * 32] = blk
    # L6: M=8 (ktile step padded to 16)
    for p, (tA, tB) in enumerate(PAIRS):
        col = OFF[6] + p * 32
        for i, tap in enumerate((tA, tB)):
            if tap is None:
                continue
            ki, kj = tap
            blk = sws[6][:, :, ki, kj].T  # (32, 2)
            for g in range(4):
                wf8[g * 32:(g + 1) * 32,
                    col + i * 16 + g * 2: col + i * 16 + (g + 1) * 2] = blk
    return w0f, wf8.astype(ml_dtypes.float8_e4m3fn)


LAST_RESULTS = None


def kernel(**inputs):
    global LAST_RESULTS
    from concourse.bass_utils import run_bass_kernel_spmd
    inp = np.asarray(inputs['inputs'], np.float32)
    ws = [np.asarray(inputs[f'w{i}']) for i in range(7)]
    w0f, wf8 = pack_weights(ws)
    nc = build_program()
    in_maps = []
    for c in range(8):
        in_maps.append({'x': np.ascontiguousarray(inp[c * 8:(c + 1) * 8]),
                        'w0f': w0f, 'wf8': wf8})
    res = run_bass_kernel_spmd(nc, in_maps, core_ids=list(range(8)),
                               tmpdir=os.environ.get('KERNEL_TRACE_DIR') or None)
    LAST_RESULTS = res
    out = np.concatenate([res.results[c]['y'] for c in range(8)], axis=0)
    return out.astype(np.float32)


# revision 7
# speedup vs baseline: 1.0879x; 1.0879x over previous
"""Trainium2 Bass kernel v2 for the 7-layer binarized CNN (nn_MCNET).

Data parallel over 8 cores (8 images each). Per core:
- L0 (3->4, fp32 input): 32 row-bands x 8 rows stacked on 128 partitions
  (block-diagonal weights), 9 accumulating fp32 matmuls; ACT Sign -> bf16,
  DVE 2x2 maxpool writes fp8 directly into A1's banded layout.
- L1..L6: fp8e4 DoubleRow matmuls. Activations live in per-layer banded
  buffers A_l: G bands x cin channels on 128 partitions, rows contiguous at
  stride 127 (127 % 16 == 15, so tap pairs (ki,kj)->(ki+1,kj+1) have ktile
  stride 128, a legal DoubleRow step). Tap-paired layers run 6 DoubleRow
  passes instead of 9; L5 (cin=64) k-splits channels across two 4320-byte
  slabs instead. Band nesting is chosen so every PSUM evacuation is an
  identity-partition ACT/DVE op (sign == clip for the even-integer sums),
  and inter-band halos are ONE contiguous partition-shifted SBUF DMA per
  layer. All DMAs issue from the SP (sync) engine -> HWDGE.
"""
import sys, os, dataclasses
sys.path.insert(0, '/opt/trn_rl_repo')
import numpy as np

CH = [(3, 4), (4, 8), (8, 16), (16, 32), (32, 64), (64, 32), (32, 2)]
HIN = [256, 127, 125, 123, 121, 119, 117]
HOUT = [h - 2 for h in HIN]
G = [32, 32, 16, 8, 4, 4, 4]         # in-bands per layer
BB = [8, 4, 8, 16, 32, 32, 32]       # nominal in-band rows
WP = 127                              # fp8 row stride (127 % 16 == 15)
SL5 = 4320                            # A5 slab stride (34*127=4318 -> pad to %16)
NIMG = 8
# tap pairs for t2 (tap-paired DoubleRow): ktile delta = 127*dki + dkj = 128
PAIRS = [((0, 0), (1, 1)), ((0, 1), (1, 2)), ((1, 0), (2, 1)),
         ((0, 2), None), ((2, 0), None), ((2, 2), None)]
# weight block column offsets in WF8
NB = [0, 12, 12, 12, 12, 9, 6]        # lhsT blocks per layer (l1..l6 used)
OFF = {}
_c = 0
for _l in range(1, 7):
    OFF[_l] = _c
    _c += NB[_l] * (256 if _l < 6 else 32)
WF8_COLS = _c
A_ROWS = [0, 6, 10, 18, 34, 0, 34]    # stored rows per band (A5 special)
A_COLS = [0] + [A_ROWS[l] * WP + 384 for l in range(1, 7)]
A_COLS[5] = 2 * SL5 + 384


def build_program():
    import concourse.bass as bass
    import concourse.mybir as mybir
    dt = mybir.dt
    AF = mybir.ActivationFunctionType
    PM = mybir.MatmulPerfMode
    ALU = mybir.AluOpType

    nc = bass.Bass("TRN2", target_bir_lowering=False)
    x = nc.dram_tensor("x", (NIMG, 3, 256, 256), dt.float32, kind="ExternalInput")
    w0f = nc.dram_tensor("w0f", (96, 9 * 128), dt.bfloat16, kind="ExternalInput")
    wf8 = nc.dram_tensor("wf8", (128, WF8_COLS), dt.float8e4, kind="ExternalInput")
    y = nc.dram_tensor("y", (NIMG, 2 * 115 * 115), dt.float32, kind="ExternalOutput")

    ctxs = []
    def alloc(cm):
        ctxs.append(cm)
        return cm.__enter__()

    WT0 = alloc(nc.sbuf_tensor("WT0", [128, 9 * 128], dt.bfloat16))
    WF8 = alloc(nc.sbuf_tensor("WF8", [128, WF8_COLS], dt.float8e4))
    A0 = alloc(nc.sbuf_tensor("A0", [128, 2 * 10 * 256], dt.float32))
    H = alloc(nc.sbuf_tensor("H", [128, 2 * 3 * 2560], dt.bfloat16))
    R1 = alloc(nc.sbuf_tensor("R1", [128, 2560], dt.float32))
    R2 = alloc(nc.sbuf_tensor("R2", [128, 2560], dt.float32))
    A = [None] * 7
    for l in range(1, 7):
        A[l] = alloc(nc.sbuf_tensor(f"A{l}", [128, A_COLS[l]], dt.float8e4))
    T0B = alloc(nc.sbuf_tensor("T0B", [128, 2032], dt.bfloat16))
    T1B = alloc(nc.sbuf_tensor("T1B", [128, 4 * 254], dt.bfloat16))
    OUTB = alloc(nc.sbuf_tensor("OUTB", [128, 32 * 115], dt.float32))
    P = [alloc(nc.psum_tensor(f"P{i}", [128, 2048], dt.float32)) for i in range(2)]
    sem = {n: alloc(nc.semaphore(name=n)) for n in
           ['sdma', 'spe', 'sact', 'sdve', 'sgp', 'sin0', 'sin1', 'swf',
            'sh1', 'sh2', 'sh3', 'sh4', 'sh5', 'sh6', 'sout']}

    def walk(E, me):
        cnt = {'dma': 0, 'pe': 0, 'act': 0, 'dve': 0, 'gp': 0}
        last_wait = {}

        def wait(eng, semn, val):
            if val is None or val <= 0:
                return
            k = (eng, semn)
            if last_wait.get(k, -1) >= val:
                return
            last_wait[k] = val
            if eng == me:
                E.wait_ge(sem[semn], val)

        def emit(eng, fn):
            if eng == me:
                return fn()
            return None

        def inc(inst, semn, v):
            if inst is not None:
                inst.then_inc(sem[semn], v)

        # ---- init: memsets ----
        # A0: only band 31's rows 8,9 are never DMA-written (both slots).
        i = emit('gp', lambda: nc.gpsimd.memset(A0[64:96, 2048:2560], 0.0))
        cnt['gp'] += 1
        inc(i, 'sgp', 1)
        i = emit('gp', lambda: nc.gpsimd.memset(A0[64:96, 4608:5120], 0.0))
        cnt['gp'] += 1
        inc(i, 'sgp', 1)
        for l in range(1, 7):
            i = emit('gp', lambda l=l:
                     nc.gpsimd.memset(A[l][0:128, 0:A_COLS[l]], 0.0))
            cnt['gp'] += 1
            inc(i, 'sgp', 1)
        NMEMSET = cnt['gp']
        MS_A = {0: 2, 1: 3, 2: 4, 3: 5, 4: 6, 5: 7, 6: 8}  # sgp count when A_l ready
        # ---- weight DMA: WT0 first (L0); WF8 deferred until after in(0/1) ----
        i = emit('sp', lambda: nc.sync.dma_start(WT0[0:96, :], w0f[:]))
        inc(i, 'sdma', 16)

        slot_free = [None, None]
        a0_free = [None, None]
        h_free = [None, None]
        vmax_prev = None            # sdve count of prev img vmax (T0B free)
        hcnt = {l: 0 for l in range(1, 7)}   # per-halo-sem cumulative counts
        out_cnt = 0
        tile_g = 0

        def pe_tile_begin(slot, waits_other=(), layer=1):
            # common PE-tile prologue: memsets + weights + input-ready + slot
            if layer == 0:
                wait('pe', 'sgp', 2)        # A0 tail memsets
                wait('pe', 'sdma', 16)      # WT0
            else:
                wait('pe', 'sgp', MS_A[layer])   # A_layer memset done
                wait('pe', 'swf', 16)       # WF8
            for sname, v in waits_other:
                wait('pe', sname, v)
            if slot_free[slot] is not None:
                wait('pe', slot_free[slot][0], slot_free[slot][1])

        def dr_matmul(PS, psoff, lhs_col, lhs_m, rhs_buf, rhs_off, rhs_delta,
                      n, start, stop, final):
            # one DoubleRow matmul; final -> inc spe
            lstep = max(16, lhs_m)
            def mk():
                lv = WF8[0:128, lhs_col: lhs_col + lstep + lhs_m]
                lv = dataclasses.replace(lv, ap=[lv.ap[0], [lstep, 2], [1, lhs_m]])
                rv = rhs_buf[0:128, rhs_off: rhs_off + rhs_delta + n]
                rv = dataclasses.replace(rv, ap=[rv.ap[0], [rhs_delta, 2], [1, n]])
                ov = PS[0:lhs_m, psoff: psoff + n]
                return nc.tensor.matmul(ov, lv, rv, start=start, stop=stop,
                                        perf_mode=PM.DoubleRow)
            i = emit('pe', mk)
            if final:
                cnt['pe'] += 1
                inc(i, 'spe', 1)
            return i

        def evac(eng, PS, nchunks, nlast, dstbuf, dstoff, dst_ms, mpart,
                 sdma_guard):
            # evacuate psum chunks [512-strided, 508(or nlast) wide] -> dst
            # contiguous; sign/clip. eng in ('act','dve').
            wait(eng, 'spe', cnt['pe'])
            if dst_ms is not None:
                wait(eng, 'sgp', dst_ms)
            if sdma_guard is not None:
                wait(eng, sdma_guard[0], sdma_guard[1])
            total = 508 * (nchunks - 1) + nlast
            def mk():
                sv = PS[0:mpart, 0:(nchunks - 1) * 512 + nlast]
                sv = dataclasses.replace(sv, ap=[sv.ap[0], [512, nchunks], [1, 508]]) \
                    if nchunks > 1 else dataclasses.replace(sv, ap=[sv.ap[0], [1, nlast]])
                dv = dstbuf[0:mpart, dstoff: dstoff + total]
                if eng == 'act':
                    return nc.scalar.activation(dv, sv, AF.Sign)
                return nc.vector.tensor_scalar(dv, sv, 1.0, -1.0, ALU.min, ALU.max)
            i = emit(eng, mk)
            key = 'sact' if eng == 'act' else 'sdve'
            cnt[eng] += 1
            inc(i, key, 1)
            return (key, cnt[eng])

        dma_in_done = [None] * NIMG
        in_cnt = [0, 0]

        def emit_in_dma(j):
            # input DMA for image j (slot j%2); issued one image ahead.
            # Dedicated per-slot semaphore so the wait value is race-free
            # (only this slot's DMAs are ever outstanding on it).
            aslot = j % 2
            sname = f'sin{aslot}'
            off = aslot * 2560
            wait('sp', 'sgp', 2)      # A0 tail memsets done
            if a0_free[aslot] is not None:
                wait('sp', a0_free[aslot][0], a0_free[aslot][1])
            src_main = dataclasses.replace(
                x[j], ap=[[2048, 31], [65536, 3], [256, 10], [1, 256]])
            i = emit('sp', lambda src_main=src_main, off=off:
                     nc.sync.dma_start(A0[0:93, off:off + 2560], src_main))
            in_cnt[aslot] += 1
            inc(i, sname, 16)
            i = emit('sp', lambda j=j, off=off:
                     nc.sync.dma_start(A0[93:96, off:off + 2048], x[j, :, 248:256, :]))
            in_cnt[aslot] += 1
            inc(i, sname, 16)
            dma_in_done[j] = (sname, in_cnt[aslot])

        decomp_done = [None] * NIMG
        a0_read = [None] * NIMG

        def emit_decomp(j):
            # split fp32 A0 slot into 3 bf16 terms in H (exact to ~2^-25):
            # runs entirely on the otherwise-idle Pool engine during the
            # previous image's compute (self-waits guard pipelined RAW).
            sl = j % 2
            aoff = sl * 2560
            hoff = sl * 7680
            wait('gp', dma_in_done[j][0], 16 * dma_in_done[j][1])
            if h_free[sl] is not None:
                wait('gp', 'spe', h_free[sl])
            i = emit('gp', lambda aoff=aoff, hoff=hoff: nc.gpsimd.tensor_copy(
                H[0:96, hoff: hoff + 2560], A0[0:96, aoff: aoff + 2560]))
            cnt['gp'] += 1
            inc(i, 'sgp', 1)
            wait('gp', 'sgp', cnt['gp'])
            i = emit('gp', lambda aoff=aoff, hoff=hoff: nc.gpsimd.tensor_tensor(
                R1[0:96, 0:2560], A0[0:96, aoff: aoff + 2560],
                H[0:96, hoff: hoff + 2560], ALU.subtract))
            cnt['gp'] += 1
            inc(i, 'sgp', 1)
            a0_read[j] = cnt['gp']
            a0_free[sl] = ('sgp', cnt['gp'])
            wait('gp', 'sgp', cnt['gp'])
            i = emit('gp', lambda hoff=hoff: nc.gpsimd.tensor_copy(
                H[0:96, hoff + 2560: hoff + 5120], R1[0:96, 0:2560]))
            cnt['gp'] += 1
            inc(i, 'sgp', 1)
            wait('gp', 'sgp', cnt['gp'])
            i = emit('gp', lambda hoff=hoff: nc.gpsimd.tensor_tensor(
                R2[0:96, 0:2560], R1[0:96, 0:2560],
                H[0:96, hoff + 2560: hoff + 5120], ALU.subtract))
            cnt['gp'] += 1
            inc(i, 'sgp', 1)
            wait('gp', 'sgp', cnt['gp'])
            i = emit('gp', lambda hoff=hoff: nc.gpsimd.tensor_copy(
                H[0:96, hoff + 5120: hoff + 7680], R2[0:96, 0:2560]))
            cnt['gp'] += 1
            inc(i, 'sgp', 1)
            decomp_done[j] = cnt['gp']

        for img in range(NIMG):
            # ======== input DMA prefetch (this img on img0, next img after) ====
            if img == 0:
                emit_in_dma(0)
            if img + 1 < NIMG:
                emit_in_dma(img + 1)
            if img == 0:
                emit_decomp(0)
                # WF8 weight DMA after the first input DMAs
                i = emit('sp', lambda: nc.sync.dma_start(WF8[0:128, :], wf8[:]))
                inc(i, 'swf', 16)
            dma_in = dma_in_done[img]
            aslot = img % 2
            off = aslot * 2560

            # ======== L0: fp32 conv, 32 bands x 8 rows ========
            slot = tile_g % 2; tile_g += 1
            PS = P[slot]
            pe_tile_begin(slot, waits_other=[('sgp', decomp_done[img])], layer=0)
            hbase = aslot * 7680
            l0_chunk_pe = []
            for c in range(4):
                for t in range(3):
                    for tap in range(9):
                        ki, kj = tap // 3, tap % 3
                        rbase = hbase + t * 2560 + (2 * c + ki) * 256 + kj
                        def mk(c=c, rbase=rbase, tap=tap, PS=PS):
                            lhsT = WT0[0:96, tap * 128: tap * 128 + 128]
                            rv = H[0:96, rbase: rbase + 256 + 254]
                            rv = dataclasses.replace(rv,
                                                     ap=[rv.ap[0], [256, 2], [1, 254]])
                            ov = PS[0:128, c * 512: c * 512 + 508]
                            return nc.tensor.matmul(
                                ov, lhsT, rv, start=(t == 0 and tap == 0),
                                stop=(t == 2 and tap == 8))
                        i = emit('pe', mk)
                        if t == 2 and tap == 8:
                            cnt['pe'] += 1
                            inc(i, 'spe', 1)
                l0_chunk_pe.append(cnt['pe'])
            h_free[aslot] = cnt['pe']
            # prefetch: decompose the NEXT image during this image's layers
            if img + 1 < NIMG:
                emit_decomp(img + 1)

            # per-chunk pool pipeline: sign(c) -> vmax(c) -> hmax(c)
            vmax_cur = []
            for c in range(4):
                wait('act', 'spe', l0_chunk_pe[c])
                if vmax_prev is not None:
                    wait('act', 'sdve', vmax_prev[c])
                def mksgn(PS=PS, c=c):
                    return nc.scalar.activation(
                        T0B[0:128, c * 508: c * 508 + 508],
                        PS[0:128, c * 512: c * 512 + 508], AF.Sign)
                i = emit('act', mksgn)
                cnt['act'] += 1
                inc(i, 'sact', 1)
                if c == 3:
                    slot_free[slot] = ('sact', cnt['act'])
                # DVE vmax(c): rows 2c,2c+1 -> T1B row c
                wait('dve', 'sact', cnt['act'])
                def mkv(c=c):
                    a = T0B[0:128, c * 508: c * 508 + 254]
                    b = T0B[0:128, c * 508 + 254: c * 508 + 508]
                    return nc.vector.tensor_max(
                        T1B[0:128, c * 254: c * 254 + 254], a, b)
                i = emit('dve', mkv)
                cnt['dve'] += 1
                inc(i, 'sdve', 1)
                vmax_cur.append(cnt['dve'])
                # DVE hmax(c) -> A1 row c
                wait('dve', 'sdve', cnt['dve'])   # RAW on T1B
                if c == 0:
                    wait('dve', 'sgp', MS_A[1])
                    wait('dve', 'sh1', hcnt[1])
                def mkh(c=c):
                    sv = T1B[0:128, c * 254: c * 254 + 254]
                    a = dataclasses.replace(sv, ap=[sv.ap[0], [2, 127]])
                    b = dataclasses.replace(sv, offset=sv.offset + 1,
                                            ap=[sv.ap[0], [2, 127]])
                    return nc.vector.tensor_max(
                        A[1][0:128, c * WP: c * WP + 127], a, b)
                i = emit('dve', mkh)
                cnt['dve'] += 1
                inc(i, 'sdve', 1)
                if c == 1:
                    hmax01 = cnt['dve']
            hmax_all = cnt['dve']
            vmax_prev = vmax_cur

            # halo A1 (SP): bands g+1 rows 0:2 -> band g rows 4:6
            wait('sp', 'sdve', hmax01)
            i = emit('sp', lambda: nc.sync.dma_start(
                A[1][0:124, 4 * WP: 6 * WP], A[1][4:128, 0:2 * WP]))
            hcnt[1] += 16
            inc(i, 'sh1', 16)
            halo1 = ('sh1', hcnt[1])

            # ======== L1: G32 -> 16 out-bands, 2 phase-chunks ========
            slot = tile_g % 2; tile_g += 1
            PS = P[slot]
            pe_tile_begin(slot, waits_other=[halo1, ('sdve', hmax_all)])
            for p, (tA, tB) in enumerate(PAIRS):
                for ph in range(2):
                    kiA, kjA = tA
                    delta = 128 if tB else 16
                    dr_matmul(PS, ph * 512, OFF[1] + (p * 2 + ph) * 256, 128,
                              A[1], kiA * WP + kjA, delta, 508,
                              start=(p == 0), stop=(p == 5),
                              final=(p == 5 and ph == 1))
            # evac (DVE) -> A2 rows 0..8
            efree = evac('dve', PS, 2, 508, A[2], 0, MS_A[2], 128,
                         ('sh2', hcnt[2]) if hcnt[2] else None)
            l1_evac = efree
            slot_free[slot] = efree
            # halo A2
            wait('sp', efree[0], efree[1])
            i = emit('sp', lambda: nc.sync.dma_start(
                A[2][0:120, 8 * WP: 10 * WP], A[2][8:128, 0:2 * WP]))
            hcnt[2] += 16
            inc(i, 'sh2', 16)
            halo2 = ('sh2', hcnt[2])

            # ======== L2: G16 -> 8, chunks (ph, j) ========
            slot = tile_g % 2; tile_g += 1
            PS = P[slot]
            pe_tile_begin(slot, waits_other=[l1_evac])
            for j in range(2):
                if j == 1:
                    wait('pe', halo2[0], halo2[1])
                for ph in range(2):
                    for p, (tA, tB) in enumerate(PAIRS):
                        kiA, kjA = tA
                        delta = 128 if tB else 16
                        dr_matmul(PS, (2 * ph + j) * 512,
                                  OFF[2] + (p * 2 + ph) * 256, 128,
                                  A[2], (4 * j + kiA) * WP + kjA, delta, 508,
                                  start=(p == 0), stop=(p == 5),
                                  final=(p == 5 and ph == 1 and j == 1))
            efree = evac('act', PS, 4, 508, A[3], 0, MS_A[3], 128,
                         ('sh3', hcnt[3]) if hcnt[3] else None)
            l2_evac = efree
            slot_free[slot] = efree
            wait('sp', efree[0], efree[1])
            i = emit('sp', lambda: nc.sync.dma_start(
                A[3][0:112, 16 * WP: 18 * WP], A[3][16:128, 0:2 * WP]))
            hcnt[3] += 16
            inc(i, 'sh3', 16)
            halo3 = ('sh3', hcnt[3])

            # ======== L3: G8 -> 4, two row-tiles (tau = phase) ========
            l3_evacs = []
            for tau in range(2):
                slot = tile_g % 2; tile_g += 1
                PS = P[slot]
                pe_tile_begin(slot, waits_other=[l2_evac])
                for j in range(4):
                    if j == 3:
                        wait('pe', halo3[0], halo3[1])
                    for p, (tA, tB) in enumerate(PAIRS):
                        kiA, kjA = tA
                        delta = 128 if tB else 16
                        dr_matmul(PS, j * 512, OFF[3] + (p * 2 + tau) * 256, 128,
                                  A[3], (4 * j + kiA) * WP + kjA, delta, 508,
                                  start=(p == 0), stop=(p == 5),
                                  final=(p == 5 and j == 3))
                efree = evac('dve', PS, 4, 508, A[4], tau * 16 * WP, MS_A[4], 128,
                             ('sh4', hcnt[4]) if (tau == 0 and hcnt[4]) else None)
                slot_free[slot] = efree
                l3_evacs.append(efree)
                if tau == 0:
                    wait('sp', efree[0], efree[1])
                    i = emit('sp', lambda: nc.sync.dma_start(
                        A[4][0:96, 32 * WP: 34 * WP], A[4][32:128, 0:2 * WP]))
                    hcnt[4] += 16
                    inc(i, 'sh4', 16)
                    halo4 = ('sh4', hcnt[4])

            # ======== L4: G4, tiles (h, tau) ========
            l4_t0_evacs = []
            l4_evac_last = None
            for h in range(2):
                for tau in range(2):
                    slot = tile_g % 2; tile_g += 1
                    PS = P[slot]
                    pe_tile_begin(slot, waits_other=[l3_evacs[tau]])
                    for j in range(4):
                        if j == 3:
                            if tau == 0:
                                wait('pe', l3_evacs[1][0], l3_evacs[1][1])
                            else:
                                wait('pe', halo4[0], halo4[1])
                        for p, (tA, tB) in enumerate(PAIRS):
                            kiA, kjA = tA
                            delta = 128 if tB else 16
                            dr_matmul(PS, j * 512, OFF[4] + (p * 2 + h) * 256, 128,
                                      A[4], (16 * tau + 4 * j + kiA) * WP + kjA,
                                      delta, 508,
                                      start=(p == 0), stop=(p == 5),
                                      final=(p == 5 and j == 3))
                    efree = evac('act', PS, 4, 508, A[5],
                                 h * SL5 + tau * 16 * WP, MS_A[5], 128,
                                 ('sh5', hcnt[5]) if (h == 0 and tau == 0 and hcnt[5]) else None)
                    slot_free[slot] = efree
                    l4_evac_last = efree
                    if tau == 0:
                        l4_t0_evacs.append(efree)
            # halo A5 (both slabs, 1 DMA) after (h0,t0) and (h1,t0) evacs
            wait('sp', l4_t0_evacs[1][0], l4_t0_evacs[1][1])
            def mkh5():
                sv = A[5][32:128, 0:SL5 + 2 * WP]
                sv = dataclasses.replace(sv, ap=[sv.ap[0], [SL5, 2], [1, 2 * WP]])
                dv = A[5][0:96, 32 * WP: SL5 + 34 * WP]
                dv = dataclasses.replace(dv, ap=[dv.ap[0], [SL5, 2], [1, 2 * WP]])
                return nc.sync.dma_start(dv, sv)
            i = emit('sp', mkh5)
            hcnt[5] += 16
            inc(i, 'sh5', 16)
            halo5 = ('sh5', hcnt[5])

            # ======== L5: G4, t1 (k-split slabs), 9 taps, two row-tiles ========
            l5_evacs = []
            for tau in range(2):
                slot = tile_g % 2; tile_g += 1
                PS = P[slot]
                if tau == 0:
                    pe_tile_begin(slot, waits_other=[l4_t0_evacs[1]])
                else:
                    pe_tile_begin(slot, waits_other=[l4_evac_last])
                for j in range(4):
                    if j == 3:
                        if tau == 0:
                            wait('pe', l4_evac_last[0], l4_evac_last[1])
                        else:
                            wait('pe', halo5[0], halo5[1])
                    for tap in range(9):
                        ki, kj = tap // 3, tap % 3
                        dr_matmul(PS, j * 512, OFF[5] + tap * 256, 128,
                                  A[5], (16 * tau + 4 * j + ki) * WP + kj,
                                  SL5, 508,
                                  start=(tap == 0), stop=(tap == 8),
                                  final=(tap == 8 and j == 3))
                efree = evac('dve', PS, 4, 508, A[6], tau * 16 * WP, MS_A[6], 128,
                             ('sh6', hcnt[6]) if (tau == 0 and hcnt[6]) else None)
                slot_free[slot] = efree
                l5_evacs.append(efree)
                if tau == 0:
                    wait('sp', efree[0], efree[1])
                    i = emit('sp', lambda: nc.sync.dma_start(
                        A[6][0:96, 32 * WP: 34 * WP], A[6][32:128, 0:2 * WP]))
                    hcnt[6] += 16
                    inc(i, 'sh6', 16)
                    halo6 = ('sh6', hcnt[6])

            # ======== L6: G4, M=8, two row-tiles ========
            l6_evacs = []
            for tau in range(2):
                slot = tile_g % 2; tile_g += 1
                PS = P[slot]
                pe_tile_begin(slot, waits_other=[l5_evacs[tau]])
                for j in range(4):
                    if j == 3:
                        if tau == 0:
                            wait('pe', l5_evacs[1][0], l5_evacs[1][1])
                        else:
                            wait('pe', halo6[0], halo6[1])
                    for p, (tA, tB) in enumerate(PAIRS):
                        kiA, kjA = tA
                        delta = 128 if tB else 16
                        dr_matmul(PS, j * 512, OFF[6] + p * 32, 8,
                                  A[6], (16 * tau + 4 * j + kiA) * WP + kjA,
                                  delta, 508,
                                  start=(p == 0), stop=(p == 5),
                                  final=(p == 5 and j == 3))
                wait('act', 'spe', cnt['pe'])
                if tau == 0 and out_cnt:
                    wait('act', 'sout', out_cnt)
                def mk6(PS=PS, tau=tau):
                    sv = PS[0:8, 0:3 * 512 + 508]
                    sv = dataclasses.replace(
                        sv, ap=[sv.ap[0], [512, 4], [127, 4], [1, 115]])
                    dv = OUTB[0:8, tau * 16 * 115: tau * 16 * 115 + 16 * 115]
                    return nc.scalar.activation(dv, sv, AF.Sign)
                i = emit('act', mk6)
                cnt['act'] += 1
                inc(i, 'sact', 1)
                efree = ('sact', cnt['act'])
                slot_free[slot] = efree
                l6_evacs.append(efree)

            # ======== output DMA: 8 flat per-(band,ch) planes ========
            wait('sp', l6_evacs[1][0], l6_evacs[1][1])
            for g in range(4):
                nrows = 32 if g < 3 else 19
                for c in range(2):
                    def mko(img=img, g=g, c=c, nrows=nrows):
                        sv = OUTB[2 * g + c: 2 * g + c + 1, 0: nrows * 115]
                        dv = y[img, 0:1]
                        dv = dataclasses.replace(
                            dv, offset=dv.offset + c * 13225 + 32 * g * 115,
                            ap=[[1, nrows * 115]])
                        return nc.sync.dma_start(dv, sv)
                    i = emit('sp', mko)
                    out_cnt += 16
                    inc(i, 'sout', 16)
        return cnt

    with nc.Block() as block:
        @block.tensor
        def _(E):
            walk(E, 'pe')

        @block.scalar
        def _(E):
            walk(E, 'act')

        @block.vector
        def _(E):
            walk(E, 'dve')

        @block.gpsimd
        def _(E):
            walk(E, 'gp')

        @block.sync
        def _(E):
            walk(E, 'sp')

    for cm in reversed(ctxs):
        cm.__exit__(None, None, None)
    return nc


def pack_weights(ws):
    """ws: 7 raw arrays (cout, cin, 3, 3) -> (w0f fp32, wf8 fp8)."""
    import ml_dtypes
    sws = [np.sign(w).astype(np.float32) for w in ws]
    # L0: 32 bands x 3cin on 96 partitions -> 128 out (32 bands x 4)
    w0f = np.zeros((96, 9 * 128), np.float32)
    for tap in range(9):
        ki, kj = tap // 3, tap % 3
        blk = sws[0][:, :, ki, kj].T  # (cin, cout)
        for s in range(32):
            w0f[s * 3:s * 3 + 3, tap * 128 + s * 4: tap * 128 + s * 4 + 4] = blk
    w0f = w0f.astype(ml_dtypes.bfloat16)
    wf8 = np.zeros((128, WF8_COLS), np.float32)
    # t2 layers: 1,2,3 (phases), 4 (cout halves), 6 (plain)
    for l, nph in ((1, 2), (2, 2), (3, 2)):
        cin, cout = CH[l]
        gin = G[l]
        M = 128
        for p, (tA, tB) in enumerate(PAIRS):
            for ph in range(nph):
                col = OFF[l] + (p * 2 + ph) * 256
                for i, tap in enumerate((tA, tB)):
                    if tap is None:
                        continue
                    ki, kj = tap
                    blk = sws[l][:, :, ki, kj].T  # (cin, cout)
                    for gp_ in range(gin // 2):
                        g = 2 * gp_ + ph
                        wf8[g * cin:(g + 1) * cin,
                            col + i * M + gp_ * cout: col + i * M + (gp_ + 1) * cout] = blk
    # L4: cout halves
    cin, cout = CH[4]
    for p, (tA, tB) in enumerate(PAIRS):
        for h in range(2):
            col = OFF[4] + (p * 2 + h) * 256
            for i, tap in enumerate((tA, tB)):
                if tap is None:
                    continue
                ki, kj = tap
                blk = sws[4][32 * h:32 * h + 32, :, ki, kj].T  # (32cin, 32cout)
                for g in range(4):
                    wf8[g * 32:(g + 1) * 32,
                        col + i * 128 + g * 32: col + i * 128 + (g + 1) * 32] = blk
    # L5: t1 k-split (slab i = channels 32i..32i+32)
    for tap in range(9):
        ki, kj = tap // 3, tap % 3
        col = OFF[5] + tap * 256
        for i in range(2):
            blk = sws[5][:, 32 * i:32 * i + 32, ki, kj].T  # (32cin-half, 32cout)
            for g in range(4):
                wf8[g * 32:(g + 1) * 32,
                    col + i * 128 + g * 32: col + i * 128 + (g + 1) * 32] = blk
    # L6: M=8 (ktile step padded to 16)
    for p, (tA, tB) in enumerate(PAIRS):
        col = OFF[6] + p * 32
        for i, tap in enumerate((tA, tB)):
            if tap is None:
                continue
            ki, kj = tap
            blk = sws[6][:, :, ki, kj].T  # (32, 2)
            for g in range(4):
                wf8[g * 32:(g + 1) * 32,
                    col + i * 16 + g * 2: col + i * 16 + (g + 1) * 2] = blk
    return w0f, wf8.astype(ml_dtypes.float8_e4m3fn)


LAST_RESULTS = None


def kernel(**inputs):
    global LAST_RESULTS
    from concourse.bass_utils import run_bass_kernel_spmd
    inp = np.asarray(inputs['inputs'], np.float32)
    ws = [np.asarray(inputs[f'w{i}']) for i in range(7)]
    w0f, wf8 = pack_weights(ws)
    nc = build_program()
    in_maps = []
    for c in range(8):
        in_maps.append({'x': np.ascontiguousarray(inp[c * 8:(c + 1) * 8]),
                        'w0f': w0f, 'wf8': wf8})
    res = run_bass_kernel_spmd(nc, in_maps, core_ids=list(range(8)),
                               tmpdir=os.environ.get('KERNEL_TRACE_DIR') or None)
    LAST_RESULTS = res
    out = np.concatenate([res.results[c]['y'] for c in range(8)], axis=0)
    return out.astype(np.float32)


# revision 8
# speedup vs baseline: 1.1067x; 1.0173x over previous
"""Trainium2 Bass kernel v2 for the 7-layer binarized CNN (nn_MCNET).

Data parallel over 8 cores (8 images each). Per core:
- L0 (3->4, fp32 input): 32 row-bands x 8 rows stacked on 128 partitions
  (block-diagonal weights), 9 accumulating fp32 matmuls; ACT Sign -> bf16,
  DVE 2x2 maxpool writes fp8 directly into A1's banded layout.
- L1..L6: fp8e4 DoubleRow matmuls. Activations live in per-layer banded
  buffers A_l: G bands x cin channels on 128 partitions, rows contiguous at
  stride 127 (127 % 16 == 15, so tap pairs (ki,kj)->(ki+1,kj+1) have ktile
  stride 128, a legal DoubleRow step). Tap-paired layers run 6 DoubleRow
  passes instead of 9; L5 (cin=64) k-splits channels across two 4320-byte
  slabs instead. Band nesting is chosen so every PSUM evacuation is an
  identity-partition ACT/DVE op (sign == clip for the even-integer sums),
  and inter-band halos are ONE contiguous partition-shifted SBUF DMA per
  layer. All DMAs issue from the SP (sync) engine -> HWDGE.
"""
import sys, os, dataclasses
sys.path.insert(0, '/opt/trn_rl_repo')
import numpy as np

CH = [(3, 4), (4, 8), (8, 16), (16, 32), (32, 64), (64, 32), (32, 2)]
HIN = [256, 127, 125, 123, 121, 119, 117]
HOUT = [h - 2 for h in HIN]
G = [32, 32, 16, 8, 4, 4, 4]         # in-bands per layer
BB = [8, 4, 8, 16, 32, 32, 32]       # nominal in-band rows
WP = 127                              # fp8 row stride (127 % 16 == 15)
SL5 = 4320                            # A5 slab stride (34*127=4318 -> pad to %16)
NIMG = 8
# tap pairs for t2 (tap-paired DoubleRow): ktile delta = 127*dki + dkj = 128
PAIRS = [((0, 0), (1, 1)), ((0, 1), (1, 2)), ((1, 0), (2, 1)),
         ((0, 2), None), ((2, 0), None), ((2, 2), None)]
# weight block column offsets in WF8
NB = [0, 12, 12, 12, 12, 9, 6]        # lhsT blocks per layer (l1..l6 used)
OFF = {}
_c = 0
for _l in range(1, 7):
    OFF[_l] = _c
    _c += NB[_l] * (256 if _l < 6 else 32)
WF8_COLS = _c
A_ROWS = [0, 6, 10, 18, 34, 0, 34]    # stored rows per band (A5 special)
A_COLS = [0] + [A_ROWS[l] * WP + 384 for l in range(1, 7)]
A_COLS[5] = 2 * SL5 + 384


def build_program():
    import concourse.bass as bass
    import concourse.mybir as mybir
    dt = mybir.dt
    AF = mybir.ActivationFunctionType
    PM = mybir.MatmulPerfMode
    ALU = mybir.AluOpType

    nc = bass.Bass("TRN2", target_bir_lowering=False)
    x = nc.dram_tensor("x", (NIMG, 3, 256, 256), dt.float32, kind="ExternalInput")
    w0f = nc.dram_tensor("w0f", (96, 9 * 128), dt.bfloat16, kind="ExternalInput")
    wf8 = nc.dram_tensor("wf8", (128, WF8_COLS), dt.float8e4, kind="ExternalInput")
    y = nc.dram_tensor("y", (NIMG, 2 * 115 * 115), dt.float32, kind="ExternalOutput")

    ctxs = []
    def alloc(cm):
        ctxs.append(cm)
        return cm.__enter__()

    WT0 = alloc(nc.sbuf_tensor("WT0", [128, 9 * 128], dt.bfloat16))
    WF8 = alloc(nc.sbuf_tensor("WF8", [128, WF8_COLS], dt.float8e4))
    A0 = alloc(nc.sbuf_tensor("A0", [128, 2 * 10 * 256], dt.float32))
    H = alloc(nc.sbuf_tensor("H", [128, 2 * 3 * 2560], dt.bfloat16))
    R1 = alloc(nc.sbuf_tensor("R1", [128, 2560], dt.float32))
    R2 = alloc(nc.sbuf_tensor("R2", [128, 2560], dt.float32))
    A = [None] * 7
    for l in range(1, 7):
        A[l] = alloc(nc.sbuf_tensor(f"A{l}", [128, A_COLS[l]], dt.float8e4))
    T0B = alloc(nc.sbuf_tensor("T0B", [128, 2032], dt.bfloat16))
    T1B = alloc(nc.sbuf_tensor("T1B", [128, 4 * 254], dt.bfloat16))
    OUTB = alloc(nc.sbuf_tensor("OUTB", [128, 32 * 115], dt.float32))
    P = [alloc(nc.psum_tensor(f"P{i}", [128, 2048], dt.float32)) for i in range(2)]
    sem = {n: alloc(nc.semaphore(name=n)) for n in
           ['sdma', 'spe', 'sact', 'sdve', 'sgp', 'sin0', 'sin1', 'swf',
            'sh1', 'sh2', 'sh3', 'sh4', 'sh5', 'sh6', 'sout']}

    def walk(E, me):
        cnt = {'dma': 0, 'pe': 0, 'act': 0, 'dve': 0, 'gp': 0}
        last_wait = {}

        def wait(eng, semn, val):
            if val is None or val <= 0:
                return
            k = (eng, semn)
            if last_wait.get(k, -1) >= val:
                return
            last_wait[k] = val
            if eng == me:
                E.wait_ge(sem[semn], val)

        def emit(eng, fn):
            if eng == me:
                return fn()
            return None

        def inc(inst, semn, v):
            if inst is not None:
                inst.then_inc(sem[semn], v)

        # ---- init: memsets ----
        # A0: only band 31's rows 8,9 are never DMA-written (both slots).
        i = emit('gp', lambda: nc.gpsimd.memset(A0[64:96, 2048:2560], 0.0))
        cnt['gp'] += 1
        inc(i, 'sgp', 1)
        i = emit('gp', lambda: nc.gpsimd.memset(A0[64:96, 4608:5120], 0.0))
        cnt['gp'] += 1
        inc(i, 'sgp', 1)
        for l in range(1, 7):
            i = emit('gp', lambda l=l:
                     nc.gpsimd.memset(A[l][0:128, 0:A_COLS[l]], 0.0))
            cnt['gp'] += 1
            inc(i, 'sgp', 1)
        NMEMSET = cnt['gp']
        MS_A = {0: 2, 1: 3, 2: 4, 3: 5, 4: 6, 5: 7, 6: 8}  # sgp count when A_l ready
        # ---- weight DMA: WT0 first (L0); WF8 deferred until after in(0/1) ----
        i = emit('sp', lambda: nc.sync.dma_start(WT0[0:96, :], w0f[:]))
        inc(i, 'sdma', 16)

        slot_free = [None, None]
        a0_free = [None, None]
        h_free = [None, None]
        vmax_prev = None            # sdve count of prev img vmax (T0B free)
        hcnt = {l: 0 for l in range(1, 7)}   # per-halo-sem cumulative counts
        out_cnt = 0
        tile_g = 0

        def pe_tile_begin(slot, waits_other=(), layer=1):
            # common PE-tile prologue: memsets + weights + input-ready + slot
            if layer == 0:
                wait('pe', 'sgp', 2)        # A0 tail memsets
                wait('pe', 'sdma', 16)      # WT0
            else:
                wait('pe', 'sgp', MS_A[layer])   # A_layer memset done
                wait('pe', 'swf', 16)       # WF8
            for sname, v in waits_other:
                wait('pe', sname, v)
            if slot_free[slot] is not None:
                wait('pe', slot_free[slot][0], slot_free[slot][1])

        def dr_matmul(PS, psoff, lhs_col, lhs_m, rhs_buf, rhs_off, rhs_delta,
                      n, start, stop, final):
            # one DoubleRow matmul; final -> inc spe
            lstep = max(16, lhs_m)
            def mk():
                lv = WF8[0:128, lhs_col: lhs_col + lstep + lhs_m]
                lv = dataclasses.replace(lv, ap=[lv.ap[0], [lstep, 2], [1, lhs_m]])
                rv = rhs_buf[0:128, rhs_off: rhs_off + rhs_delta + n]
                rv = dataclasses.replace(rv, ap=[rv.ap[0], [rhs_delta, 2], [1, n]])
                ov = PS[0:lhs_m, psoff: psoff + n]
                return nc.tensor.matmul(ov, lv, rv, start=start, stop=stop,
                                        perf_mode=PM.DoubleRow)
            i = emit('pe', mk)
            if final:
                cnt['pe'] += 1
                inc(i, 'spe', 1)
            return i

        def evac(eng, PS, nchunks, nlast, dstbuf, dstoff, dst_ms, mpart,
                 sdma_guard):
            # evacuate psum chunks [512-strided, 508(or nlast) wide] -> dst
            # contiguous; sign/clip. eng in ('act','dve').
            wait(eng, 'spe', cnt['pe'])
            if dst_ms is not None:
                wait(eng, 'sgp', dst_ms)
            if sdma_guard is not None:
                wait(eng, sdma_guard[0], sdma_guard[1])
            total = 508 * (nchunks - 1) + nlast
            def mk():
                sv = PS[0:mpart, 0:(nchunks - 1) * 512 + nlast]
                sv = dataclasses.replace(sv, ap=[sv.ap[0], [512, nchunks], [1, 508]]) \
                    if nchunks > 1 else dataclasses.replace(sv, ap=[sv.ap[0], [1, nlast]])
                dv = dstbuf[0:mpart, dstoff: dstoff + total]
                if eng == 'act':
                    return nc.scalar.activation(dv, sv, AF.Sign)
                return nc.vector.tensor_scalar(dv, sv, 1.0, -1.0, ALU.min, ALU.max)
            i = emit(eng, mk)
            key = 'sact' if eng == 'act' else 'sdve'
            cnt[eng] += 1
            inc(i, key, 1)
            return (key, cnt[eng])

        dma_in_done = [None] * NIMG
        in_cnt = [0, 0]

        def emit_in_dma(j):
            # input DMA for image j (slot j%2); issued one image ahead.
            # Dedicated per-slot semaphore so the wait value is race-free
            # (only this slot's DMAs are ever outstanding on it).
            aslot = j % 2
            sname = f'sin{aslot}'
            off = aslot * 2560
            wait('sp', 'sgp', 2)      # A0 tail memsets done
            if a0_free[aslot] is not None:
                for sn, v in a0_free[aslot]:
                    wait('sp', sn, v)
            src_main = dataclasses.replace(
                x[j], ap=[[2048, 31], [65536, 3], [256, 10], [1, 256]])
            i = emit('sp', lambda src_main=src_main, off=off:
                     nc.sync.dma_start(A0[0:93, off:off + 2560], src_main))
            in_cnt[aslot] += 1
            inc(i, sname, 16)
            i = emit('sp', lambda j=j, off=off:
                     nc.sync.dma_start(A0[93:96, off:off + 2048], x[j, :, 248:256, :]))
            in_cnt[aslot] += 1
            inc(i, sname, 16)
            dma_in_done[j] = (sname, in_cnt[aslot])

        decomp_done = [None] * NIMG
        a0_read = [None] * NIMG

        def _decomp_chain(eng, sname, j, c0, c1):
            # one decomposition chain over A0-slot cols [c0, c1) on engine eng
            sl = j % 2
            aoff = sl * 2560
            hoff = sl * 7680
            E = {'gp': nc.gpsimd, 'dve': nc.vector}[eng]
            cw = c1 - c0
            wait(eng, dma_in_done[j][0], 16 * dma_in_done[j][1])
            if h_free[sl] is not None:
                wait(eng, 'spe', h_free[sl])
            i = emit(eng, lambda: E.tensor_copy(
                H[0:96, hoff + c0: hoff + c1], A0[0:96, aoff + c0: aoff + c1]))
            cnt[eng] += 1
            inc(i, sname, 1)
            wait(eng, sname, cnt[eng])
            i = emit(eng, lambda: E.tensor_tensor(
                R1[0:96, c0:c1], A0[0:96, aoff + c0: aoff + c1],
                H[0:96, hoff + c0: hoff + c1], ALU.subtract))
            cnt[eng] += 1
            inc(i, sname, 1)
            sub1 = (sname, cnt[eng])
            wait(eng, sname, cnt[eng])
            i = emit(eng, lambda: E.tensor_copy(
                H[0:96, hoff + 2560 + c0: hoff + 2560 + c1], R1[0:96, c0:c1]))
            cnt[eng] += 1
            inc(i, sname, 1)
            wait(eng, sname, cnt[eng])
            i = emit(eng, lambda: E.tensor_tensor(
                R2[0:96, c0:c1], R1[0:96, c0:c1],
                H[0:96, hoff + 2560 + c0: hoff + 2560 + c1], ALU.subtract))
            cnt[eng] += 1
            inc(i, sname, 1)
            wait(eng, sname, cnt[eng])
            i = emit(eng, lambda: E.tensor_copy(
                H[0:96, hoff + 5120 + c0: hoff + 5120 + c1], R2[0:96, c0:c1]))
            cnt[eng] += 1
            inc(i, sname, 1)
            return sub1, (sname, cnt[eng])

        def emit_decomp(j):
            # split fp32 A0 slot into 3 bf16 terms in H (exact to ~2^-25).
            # img0: two parallel half-chains (Pool + DVE) to shorten startup;
            # later images: single Pool chain overlapped with prior compute.
            sl = j % 2
            if j == 0:
                s1a, d1 = _decomp_chain('gp', 'sgp', j, 0, 1280)
                s1b, d2 = _decomp_chain('dve', 'sdve', j, 1280, 2560)
                a0_free[sl] = [s1a, s1b]
                decomp_done[j] = [d1, d2]
            else:
                if j == 1:
                    # img0's DVE half-chain shares R1/R2 scratch
                    wait('gp', decomp_done[0][1][0], decomp_done[0][1][1])
                s1, d1 = _decomp_chain('gp', 'sgp', j, 0, 2560)
                a0_free[sl] = [s1]
                decomp_done[j] = [d1]

        for img in range(NIMG):
            # ======== input DMA prefetch (this img on img0, next img after) ====
            if img == 0:
                emit_in_dma(0)
            if img + 1 < NIMG:
                emit_in_dma(img + 1)
            if img == 0:
                emit_decomp(0)
                # WF8 weight DMA after the first input DMAs
                i = emit('sp', lambda: nc.sync.dma_start(WF8[0:128, :], wf8[:]))
                inc(i, 'swf', 16)
            dma_in = dma_in_done[img]
            aslot = img % 2
            off = aslot * 2560

            # ======== L0: fp32 conv, 32 bands x 8 rows ========
            slot = tile_g % 2; tile_g += 1
            PS = P[slot]
            pe_tile_begin(slot, waits_other=list(decomp_done[img]), layer=0)
            hbase = aslot * 7680
            l0_chunk_pe = []
            for c in range(4):
                for t in range(3):
                    for tap in range(9):
                        ki, kj = tap // 3, tap % 3
                        rbase = hbase + t * 2560 + (2 * c + ki) * 256 + kj
                        def mk(c=c, rbase=rbase, tap=tap, PS=PS):
                            lhsT = WT0[0:96, tap * 128: tap * 128 + 128]
                            rv = H[0:96, rbase: rbase + 256 + 254]
                            rv = dataclasses.replace(rv,
                                                     ap=[rv.ap[0], [256, 2], [1, 254]])
                            ov = PS[0:128, c * 512: c * 512 + 508]
                            return nc.tensor.matmul(
                                ov, lhsT, rv, start=(t == 0 and tap == 0),
                                stop=(t == 2 and tap == 8))
                        i = emit('pe', mk)
                        if t == 2 and tap == 8:
                            cnt['pe'] += 1
                            inc(i, 'spe', 1)
                l0_chunk_pe.append(cnt['pe'])
            h_free[aslot] = cnt['pe']
            # prefetch: decompose the NEXT image during this image's layers
            if img + 1 < NIMG:
                emit_decomp(img + 1)

            # per-chunk pool pipeline: sign(c) -> vmax(c) -> hmax(c)
            vmax_cur = []
            for c in range(4):
                wait('act', 'spe', l0_chunk_pe[c])
                if vmax_prev is not None:
                    wait('act', 'sdve', vmax_prev[c])
                def mksgn(PS=PS, c=c):
                    return nc.scalar.activation(
                        T0B[0:128, c * 508: c * 508 + 508],
                        PS[0:128, c * 512: c * 512 + 508], AF.Sign)
                i = emit('act', mksgn)
                cnt['act'] += 1
                inc(i, 'sact', 1)
                if c == 3:
                    slot_free[slot] = ('sact', cnt['act'])
                # DVE vmax(c): rows 2c,2c+1 -> T1B row c
                wait('dve', 'sact', cnt['act'])
                def mkv(c=c):
                    a = T0B[0:128, c * 508: c * 508 + 254]
                    b = T0B[0:128, c * 508 + 254: c * 508 + 508]
                    return nc.vector.tensor_max(
                        T1B[0:128, c * 254: c * 254 + 254], a, b)
                i = emit('dve', mkv)
                cnt['dve'] += 1
                inc(i, 'sdve', 1)
                vmax_cur.append(cnt['dve'])
                # DVE hmax(c) -> A1 row c
                wait('dve', 'sdve', cnt['dve'])   # RAW on T1B
                if c == 0:
                    wait('dve', 'sgp', MS_A[1])
                    wait('dve', 'sh1', hcnt[1])
                def mkh(c=c):
                    sv = T1B[0:128, c * 254: c * 254 + 254]
                    a = dataclasses.replace(sv, ap=[sv.ap[0], [2, 127]])
                    b = dataclasses.replace(sv, offset=sv.offset + 1,
                                            ap=[sv.ap[0], [2, 127]])
                    return nc.vector.tensor_max(
                        A[1][0:128, c * WP: c * WP + 127], a, b)
                i = emit('dve', mkh)
                cnt['dve'] += 1
                inc(i, 'sdve', 1)
                if c == 1:
                    hmax01 = cnt['dve']
            hmax_all = cnt['dve']
            vmax_prev = vmax_cur

            # halo A1 (SP): bands g+1 rows 0:2 -> band g rows 4:6
            wait('sp', 'sdve', hmax01)
            i = emit('sp', lambda: nc.sync.dma_start(
                A[1][0:124, 4 * WP: 6 * WP], A[1][4:128, 0:2 * WP]))
            hcnt[1] += 16
            inc(i, 'sh1', 16)
            halo1 = ('sh1', hcnt[1])

            # ======== L1: G32 -> 16 out-bands, 2 phase-chunks ========
            slot = tile_g % 2; tile_g += 1
            PS = P[slot]
            pe_tile_begin(slot, waits_other=[halo1, ('sdve', hmax_all)])
            for p, (tA, tB) in enumerate(PAIRS):
                for ph in range(2):
                    kiA, kjA = tA
                    delta = 128 if tB else 16
                    dr_matmul(PS, ph * 512, OFF[1] + (p * 2 + ph) * 256, 128,
                              A[1], kiA * WP + kjA, delta, 508,
                              start=(p == 0), stop=(p == 5),
                              final=(p == 5 and ph == 1))
            # evac (DVE) -> A2 rows 0..8
            efree = evac('dve', PS, 2, 508, A[2], 0, MS_A[2], 128,
                         ('sh2', hcnt[2]) if hcnt[2] else None)
            l1_evac = efree
            slot_free[slot] = efree
            # halo A2
            wait('sp', efree[0], efree[1])
            i = emit('sp', lambda: nc.sync.dma_start(
                A[2][0:120, 8 * WP: 10 * WP], A[2][8:128, 0:2 * WP]))
            hcnt[2] += 16
            inc(i, 'sh2', 16)
            halo2 = ('sh2', hcnt[2])

            # ======== L2: G16 -> 8, chunks (ph, j) ========
            slot = tile_g % 2; tile_g += 1
            PS = P[slot]
            pe_tile_begin(slot, waits_other=[l1_evac])
            for j in range(2):
                if j == 1:
                    wait('pe', halo2[0], halo2[1])
                for ph in range(2):
                    for p, (tA, tB) in enumerate(PAIRS):
                        kiA, kjA = tA
                        delta = 128 if tB else 16
                        dr_matmul(PS, (2 * ph + j) * 512,
                                  OFF[2] + (p * 2 + ph) * 256, 128,
                                  A[2], (4 * j + kiA) * WP + kjA, delta, 508,
                                  start=(p == 0), stop=(p == 5),
                                  final=(p == 5 and ph == 1 and j == 1))
            # split evac: rows 0..8 first so L3's first chunk can start early
            eh1 = evac('act', PS, 2, 508, A[3], 0, MS_A[3], 128,
                       ('sh3', hcnt[3]) if hcnt[3] else None)
            l2_evac_h1 = eh1
            def mk_l2e2(PS=PS):
                sv = PS[0:128, 2 * 512: 3 * 512 + 508]
                sv = dataclasses.replace(sv, ap=[sv.ap[0], [512, 2], [1, 508]])
                dv = A[3][0:128, 8 * WP: 16 * WP]
                return nc.scalar.activation(dv, sv, AF.Sign)
            i = emit('act', mk_l2e2)
            cnt['act'] += 1
            inc(i, 'sact', 1)
            efree = ('sact', cnt['act'])
            l2_evac = efree
            slot_free[slot] = efree
            wait('sp', efree[0], efree[1])
            i = emit('sp', lambda: nc.sync.dma_start(
                A[3][0:112, 16 * WP: 18 * WP], A[3][16:128, 0:2 * WP]))
            hcnt[3] += 16
            inc(i, 'sh3', 16)
            halo3 = ('sh3', hcnt[3])

            # ======== L3: G8 -> 4, two row-tiles (tau = phase) ========
            l3_evacs = []
            for tau in range(2):
                slot = tile_g % 2; tile_g += 1
                PS = P[slot]
                pe_tile_begin(slot, waits_other=[l2_evac_h1])
                for j in range(4):
                    if j == 1:
                        wait('pe', l2_evac[0], l2_evac[1])
                    if j == 3:
                        wait('pe', halo3[0], halo3[1])
                    for p, (tA, tB) in enumerate(PAIRS):
                        kiA, kjA = tA
                        delta = 128 if tB else 16
                        dr_matmul(PS, j * 512, OFF[3] + (p * 2 + tau) * 256, 128,
                                  A[3], (4 * j + kiA) * WP + kjA, delta, 508,
                                  start=(p == 0), stop=(p == 5),
                                  final=(p == 5 and j == 3))
                efree = evac('dve', PS, 4, 508, A[4], tau * 16 * WP, MS_A[4], 128,
                             ('sh4', hcnt[4]) if (tau == 0 and hcnt[4]) else None)
                slot_free[slot] = efree
                l3_evacs.append(efree)
                if tau == 0:
                    wait('sp', efree[0], efree[1])
                    i = emit('sp', lambda: nc.sync.dma_start(
                        A[4][0:96, 32 * WP: 34 * WP], A[4][32:128, 0:2 * WP]))
                    hcnt[4] += 16
                    inc(i, 'sh4', 16)
                    halo4 = ('sh4', hcnt[4])

            # ======== L4: G4, tiles (h, tau) ========
            l4_t0_evacs = []
            l4_evac_last = None
            for h in range(2):
                for tau in range(2):
                    slot = tile_g % 2; tile_g += 1
                    PS = P[slot]
                    pe_tile_begin(slot, waits_other=[l3_evacs[tau]])
                    for j in range(4):
                        if j == 3:
                            if tau == 0:
                                wait('pe', l3_evacs[1][0], l3_evacs[1][1])
                            else:
                                wait('pe', halo4[0], halo4[1])
                        for p, (tA, tB) in enumerate(PAIRS):
                            kiA, kjA = tA
                            delta = 128 if tB else 16
                            dr_matmul(PS, j * 512, OFF[4] + (p * 2 + h) * 256, 128,
                                      A[4], (16 * tau + 4 * j + kiA) * WP + kjA,
                                      delta, 508,
                                      start=(p == 0), stop=(p == 5),
                                      final=(p == 5 and j == 3))
                    efree = evac('act', PS, 4, 508, A[5],
                                 h * SL5 + tau * 16 * WP, MS_A[5], 128,
                                 ('sh5', hcnt[5]) if (h == 0 and tau == 0 and hcnt[5]) else None)
                    slot_free[slot] = efree
                    l4_evac_last = efree
                    if tau == 0:
                        l4_t0_evacs.append(efree)
            # halo A5 (both slabs, 1 DMA) after (h0,t0) and (h1,t0) evacs
            wait('sp', l4_t0_evacs[1][0], l4_t0_evacs[1][1])
            def mkh5():
                sv = A[5][32:128, 0:SL5 + 2 * WP]
                sv = dataclasses.replace(sv, ap=[sv.ap[0], [SL5, 2], [1, 2 * WP]])
                dv = A[5][0:96, 32 * WP: SL5 + 34 * WP]
                dv = dataclasses.replace(dv, ap=[dv.ap[0], [SL5, 2], [1, 2 * WP]])
                return nc.sync.dma_start(dv, sv)
            i = emit('sp', mkh5)
            hcnt[5] += 16
            inc(i, 'sh5', 16)
            halo5 = ('sh5', hcnt[5])

            # ======== L5: G4, t1 (k-split slabs), 9 taps, two row-tiles ========
            l5_evacs = []
            for tau in range(2):
                slot = tile_g % 2; tile_g += 1
                PS = P[slot]
                if tau == 0:
                    pe_tile_begin(slot, waits_other=[l4_t0_evacs[1]])
                else:
                    pe_tile_begin(slot, waits_other=[l4_evac_last])
                for j in range(4):
                    if j == 3:
                        if tau == 0:
                            wait('pe', l4_evac_last[0], l4_evac_last[1])
                        else:
                            wait('pe', halo5[0], halo5[1])
                    for tap in range(9):
                        ki, kj = tap // 3, tap % 3
                        dr_matmul(PS, j * 512, OFF[5] + tap * 256, 128,
                                  A[5], (16 * tau + 4 * j + ki) * WP + kj,
                                  SL5, 508,
                                  start=(tap == 0), stop=(tap == 8),
                                  final=(tap == 8 and j == 3))
                efree = evac('dve', PS, 4, 508, A[6], tau * 16 * WP, MS_A[6], 128,
                             ('sh6', hcnt[6]) if (tau == 0 and hcnt[6]) else None)
                slot_free[slot] = efree
                l5_evacs.append(efree)
                if tau == 0:
                    wait('sp', efree[0], efree[1])
                    i = emit('sp', lambda: nc.sync.dma_start(
                        A[6][0:96, 32 * WP: 34 * WP], A[6][32:128, 0:2 * WP]))
                    hcnt[6] += 16
                    inc(i, 'sh6', 16)
                    halo6 = ('sh6', hcnt[6])

            # ======== L6: G4, M=8, two row-tiles ========
            l6_evacs = []
            for tau in range(2):
                slot = tile_g % 2; tile_g += 1
                PS = P[slot]
                pe_tile_begin(slot, waits_other=[l5_evacs[tau]])
                for j in range(4):
                    if j == 3:
                        if tau == 0:
                            wait('pe', l5_evacs[1][0], l5_evacs[1][1])
                        else:
                            wait('pe', halo6[0], halo6[1])
                    for p, (tA, tB) in enumerate(PAIRS):
                        kiA, kjA = tA
                        delta = 128 if tB else 16
                        dr_matmul(PS, j * 512, OFF[6] + p * 32, 8,
                                  A[6], (16 * tau + 4 * j + kiA) * WP + kjA,
                                  delta, 508,
                                  start=(p == 0), stop=(p == 5),
                                  final=(p == 5 and j == 3))
                wait('act', 'spe', cnt['pe'])
                if tau == 0 and out_cnt:
                    wait('act', 'sout', out_cnt)
                def mk6(PS=PS, tau=tau):
                    sv = PS[0:8, 0:3 * 512 + 508]
                    sv = dataclasses.replace(
                        sv, ap=[sv.ap[0], [512, 4], [127, 4], [1, 115]])
                    dv = OUTB[0:8, tau * 16 * 115: tau * 16 * 115 + 16 * 115]
                    return nc.scalar.activation(dv, sv, AF.Sign)
                i = emit('act', mk6)
                cnt['act'] += 1
                inc(i, 'sact', 1)
                efree = ('sact', cnt['act'])
                slot_free[slot] = efree
                l6_evacs.append(efree)

            # ======== output DMA: 8 flat per-(band,ch) planes ========
            wait('sp', l6_evacs[1][0], l6_evacs[1][1])
            for g in range(4):
                nrows = 32 if g < 3 else 19
                for c in range(2):
                    def mko(img=img, g=g, c=c, nrows=nrows):
                        sv = OUTB[2 * g + c: 2 * g + c + 1, 0: nrows * 115]
                        dv = y[img, 0:1]
                        dv = dataclasses.replace(
                            dv, offset=dv.offset + c * 13225 + 32 * g * 115,
                            ap=[[1, nrows * 115]])
                        return nc.sync.dma_start(dv, sv)
                    i = emit('sp', mko)
                    out_cnt += 16
                    inc(i, 'sout', 16)
        return cnt

    with nc.Block() as block:
        @block.tensor
        def _(E):
            walk(E, 'pe')

        @block.scalar
        def _(E):
            walk(E, 'act')

        @block.vector
        def _(E):
            walk(E, 'dve')

        @block.gpsimd
        def _(E):
            walk(E, 'gp')

        @block.sync
        def _(E):
            walk(E, 'sp')

    for cm in reversed(ctxs):
        cm.__exit__(None, None, None)
    return nc


def pack_weights(ws):
    """ws: 7 raw arrays (cout, cin, 3, 3) -> (w0f fp32, wf8 fp8)."""
    import ml_dtypes
    sws = [np.sign(w).astype(np.float32) for w in ws]
    # L0: 32 bands x 3cin on 96 partitions -> 128 out (32 bands x 4)
    w0f = np.zeros((96, 9 * 128), np.float32)
    for tap in range(9):
        ki, kj = tap // 3, tap % 3
        blk = sws[0][:, :, ki, kj].T  # (cin, cout)
        for s in range(32):
            w0f[s * 3:s * 3 + 3, tap * 128 + s * 4: tap * 128 + s * 4 + 4] = blk
    w0f = w0f.astype(ml_dtypes.bfloat16)
    wf8 = np.zeros((128, WF8_COLS), np.float32)
    # t2 layers: 1,2,3 (phases), 4 (cout halves), 6 (plain)
    for l, nph in ((1, 2), (2, 2), (3, 2)):
        cin, cout = CH[l]
        gin = G[l]
        M = 128
        for p, (tA, tB) in enumerate(PAIRS):
            for ph in range(nph):
                col = OFF[l] + (p * 2 + ph) * 256
                for i, tap in enumerate((tA, tB)):
                    if tap is None:
                        continue
                    ki, kj = tap
                    blk = sws[l][:, :, ki, kj].T  # (cin, cout)
                    for gp_ in range(gin // 2):
                        g = 2 * gp_ + ph
                        wf8[g * cin:(g + 1) * cin,
                            col + i * M + gp_ * cout: col + i * M + (gp_ + 1) * cout] = blk
    # L4: cout halves
    cin, cout = CH[4]
    for p, (tA, tB) in enumerate(PAIRS):
        for h in range(2):
            col = OFF[4] + (p * 2 + h) * 256
            for i, tap in enumerate((tA, tB)):
                if tap is None:
                    continue
                ki, kj = tap
                blk = sws[4][32 * h:32 * h + 32, :, ki, kj].T  # (32cin, 32cout)
                for g in range(4):
                    wf8[g * 32:(g + 1) * 32,
                        col + i * 128 + g * 32: col + i * 128 + (g + 1) * 32] = blk
    # L5: t1 k-split (slab i = channels 32i..32i+32)
    for tap in range(9):
        ki, kj = tap // 3, tap % 3
        col = OFF[5] + tap * 256
        for i in range(2):
            blk = sws[5][:, 32 * i:32 * i + 32, ki, kj].T  # (32cin-half, 32cout)
            for g in range(4):
                wf8[g * 32:(g + 1) * 32,
                    col + i * 128 + g * 32: col + i * 128 + (g + 1) * 32] = blk
    # L6: M=8 (ktile step padded to 16)
    for p, (tA, tB) in enumerate(PAIRS):
        col = OFF[6] + p * 32
        for i, tap in enumerate((tA, tB)):
            if tap is None:
                continue
            ki, kj = tap
            blk = sws[6][:, :, ki, kj].T  # (32, 2)
            for g in range(4):
                wf8[g * 32:(g + 1) * 32,
                    col + i * 16 + g * 2: col + i * 16 + (g + 1) * 2] = blk
    return w0f, wf8.astype(ml_dtypes.float8_e4m3fn)


LAST_RESULTS = None


def kernel(**inputs):
    global LAST_RESULTS
    from concourse.bass_utils import run_bass_kernel_spmd
    inp = np.asarray(inputs['inputs'], np.float32)
    ws = [np.asarray(inputs[f'w{i}']) for i in range(7)]
    w0f, wf8 = pack_weights(ws)
    nc = build_program()
    in_maps = []
    for c in range(8):
        in_maps.append({'x': np.ascontiguousarray(inp[c * 8:(c + 1) * 8]),
                        'w0f': w0f, 'wf8': wf8})
    res = run_bass_kernel_spmd(nc, in_maps, core_ids=list(range(8)),
                               tmpdir=os.environ.get('KERNEL_TRACE_DIR') or None)
    LAST_RESULTS = res
    out = np.concatenate([res.results[c]['y'] for c in range(8)], axis=0)
    return out.astype(np.float32)
